# revision 51
# baseline (speedup 1.0000x reference)
"""Trainium2 Bass kernel for EquivariantEdgeAttention (E=384, HID=128, NH=8).

Sharding: 8 cores, core c computes query-edge rows [48c, 48c+48) of the
pairwise attention; params / keys / values / coords are replicated.
Outputs are gathered by concatenation on the host. No collectives.

Self-contained: only numpy + concourse (bass) imports, no sibling files.
"""

import numpy as np

E = 384
NC = 8
I = E // NC          # 48 query rows per core
HID = 128
NH = 8
DH = 16
NR = 64
CUTOFF = 10.0

# ---- tuning config ----
B2 = 6               # i-block size in phase 2 (z2 psum banks per block)
B1 = 12              # i-block size in phase 1 (rbf/base build)
ACT_DT = None        # filled below (bf16) - dtype of x2 / W3diag path
BASE_BF16 = True     # store base (rbf+dot proj) as bf16
X1_FP32 = True       # x1 (L2 rhs) and W2 in fp32 (vs bf16)
MM_F32R = True      # use float32r for big fp32 matmuls


# --------------------------------------------------------------------------
# walrus workaround: this container's walrus rejects >1 sync wait on a CTRL
# Drain.  Split the TileContext tail drain into single-wait drains.
# --------------------------------------------------------------------------
def _patch_tile_drain():
    import concourse.tile as tile
    from concourse import mybir
    from concourse.vector_clock import ScopedClock

    def _drain_and_barrier_split(self, tick_clock, wait_clock):
        nc = self.nc
        drain_inst = nc.sync.drain()
        wait_clock.add_sem_waits(
            drain_inst.ins, ScopedClock({None: tick_clock.global_clock})
        )
        si = drain_inst.ins.sync_info
        waits = list(si.on_wait or [])
        if len(waits) > 1:
            drain_inst.ins.sync_info = mybir.SyncInfo(
                on_wait=waits[:1], on_update=list(si.on_update or [])
            )
            for w in waits[1:]:
                d2 = nc.sync.drain()
                d2.ins.sync_info = mybir.SyncInfo(on_wait=[w], on_update=[])
        nc.all_engine_barrier()
        assert self.sems is not None
        popped = nc._tile_sem_poison_stack.pop()
        assert popped is self._sem_poison
        nc.clear_and_free_semaphores(list(self.sems.allocated().values()))
        nc.all_engine_barrier()

    tile.TileContext._drain_and_barrier = _drain_and_barrier_split


def _split_excess_waits(nc, max_waits=1):
    """This container's walrus supports only one sync-wait command per
    engine instruction.  Move excess waits onto single-wait NoOps inserted
    immediately before the instruction on the same engine.  DMA-class
    instructions keep their waits (queue-mediated, not engine-decoded)."""
    from concourse import mybir

    n = 0
    for f in nc.m.functions:
        for bb in f.blocks:
            out = []
            for inst in bb.instructions:
                tn = type(inst).__name__
                si = inst.sync_info
                waits = list(si.on_wait) if si and si.on_wait else []
                if len(waits) > max_waits:
                    for w in waits[:-max_waits]:
                        nop = mybir.InstNoOp(
                            name=f"{inst.name}-w{n}", ins=[], outs=[],
                            engine=inst.engine)
                        nop.sync_info = mybir.SyncInfo(on_wait=[w], on_update=[])
                        out.append(nop)
                        n += 1
                    inst.sync_info = mybir.SyncInfo(
                        on_wait=waits[-max_waits:],
                        on_update=list(si.on_update or []))
                out.append(inst)
            bb.instructions = out
    return n


def build_program():
    import concourse.bass as bass
    import concourse.tile as tile
    from concourse import mybir
    from contextlib import ExitStack

    _patch_tile_drain()

    f32 = mybir.dt.float32
    bf16 = mybir.dt.bfloat16
    AF = mybir.ActivationFunctionType
    OP = mybir.AluOpType
    AX = mybir.AxisListType

    act_dt = bf16                      # x2 / W3diag dtype
    f32r = mybir.dt.float32r
    mm_f32 = f32r if MM_F32R else f32   # dtype of fp32 tensors feeding matmuls
    x1_dt = mm_f32 if X1_FP32 else bf16   # x1 / W2 dtype
    base_dt = bf16 if BASE_BF16 else f32

    nc = bass.Bass("TRN2", num_devices=NC)

    def din(name, shape, dt=f32):
        return nc.declare_dram_parameter(name, list(shape), dt, isOutput=False)

    # ---------------- dram inputs ----------------
    XT = din("XT", [HID, E], mm_f32)       # features^T (replicated)
    XST = din("XST", [HID, I], mm_f32)     # slab features^T
    FS = din("FS", [I, HID])              # slab features (residual)
    CT = din("CT", [3, E])                # coords^T
    CS = din("CS", [3, I])                # slab coords^T
    CE = din("CE", [E, 3])                # coords natural
    CSL = din("CSL", [I, 3])              # slab coords natural
    MASK = din("MASK", [I, E], mm_f32)
    EYE48R = din("EYE48R", [I, I], mm_f32)
    WQ = din("WQ", [HID, HID], mm_f32)
    WK = din("WK", [HID, HID], mm_f32)
    WV = din("WV", [HID, HID], mm_f32)
    W1Q = din("W1Q", [DH, HID], mm_f32)
    W1K = din("W1K", [DH, HID], mm_f32)
    W1R = din("W1R", [NR, HID], mm_f32)   # a_w1 rbf rows
    W1D3 = din("W1D3", [3, HID])          # a_w1 dot row replicated x3
    CT3R = din("CT3R", [3, E], mm_f32)    # coords^T for dot-term matmul
    B1C = din("B1C", [HID, 1])
    W2 = din("W2", [HID, HID], x1_dt)
    B2C = din("B2C", [HID, 1])
    W3C = din("W3C", [HID, NH], act_dt)   # a_w3
    NEGC = din("NEGC", [NR, 1])           # -(centers - 1e-8)
    NEGW = din("NEGW", [NR, 1])           # -widths
    GW1 = din("GW1", [DH, HID])
    GB1 = din("GB1", [HID, 1])
    GW2 = din("GW2", [HID, 1])
    GB2 = din("GB2", [1, 1])
    WO = din("WO", [HID, HID])
    BOR = din("BOR", [1, HID])
    LNG = din("LNG", [HID, 1])
    LNB = din("LNB", [HID, 1])
    EYE48 = din("EYE48", [I, I])

    OUTFT = nc.declare_dram_parameter("OUTFT", [HID, I], f32, isOutput=True)
    OUTC = nc.declare_dram_parameter("OUTC", [I, 3], f32, isOutput=True)

    DB_DRAM = nc.dram_tensor("db_scratch", [I, E], f32)

    def bcast_free(ap, n):
        """Insert a stride-0 dim of size n after the partition dim: [P, F] -> [P, n, F]."""
        return bass.AP(tensor=ap.tensor, offset=ap.offset,
                       ap=[ap.ap[0], [0, n]] + list(ap.ap[1:]))

    with tile.TileContext(nc) as tc, ExitStack() as ctx:
        singles = ctx.enter_context(tc.tile_pool(name="singles", bufs=1))

        def load(dram, shape, dt=f32, name=None):
            t = singles.tile(list(shape), dt, tag=name or dram.name)
            nc.sync.dma_start(out=t, in_=dram[:])
            return t

        # ---------------- load constants ----------------
        xt = load(XT, [HID, E], mm_f32)
        xst = load(XST, [HID, I], mm_f32)
        fs = load(FS, [I, HID])
        ct = load(CT, [3, E])
        cs = load(CS, [3, I])
        ce = []
        for c in range(3):
            t = singles.tile([128, 3], f32, tag=f"ce{c}")
            nc.sync.dma_start(out=t, in_=CE[c * 128:(c + 1) * 128, :])
            ce.append(t)
        csl = load(CSL, [I, 3])
        mask = load(MASK, [I, E], mm_f32)
        eye48r = load(EYE48R, [I, I], mm_f32)
        wq = load(WQ, [HID, HID], mm_f32)
        wk = load(WK, [HID, HID], mm_f32)
        wv = load(WV, [HID, HID], mm_f32)
        w1q = load(W1Q, [DH, HID], mm_f32)
        w1k = load(W1K, [DH, HID], mm_f32)
        w1r = load(W1R, [NR, HID], mm_f32)
        w1d3 = load(W1D3, [3, HID])
        ct3r = load(CT3R, [3, E], mm_f32)
        b1c = load(B1C, [HID, 1])
        w2 = load(W2, [HID, HID], x1_dt)
        b2c = load(B2C, [HID, 1])
        negc = load(NEGC, [NR, 1])
        negw = load(NEGW, [NR, 1])
        gw1 = load(GW1, [DH, HID])
        gb1 = load(GB1, [HID, 1])
        gw2 = load(GW2, [HID, 1])
        gb2 = load(GB2, [1, 1])
        one11 = singles.tile([1, 1], f32, tag="one11id")
        nc.vector.memset(one11, 1.0)
        wo = load(WO, [HID, HID])
        bor = load(BOR, [1, HID])
        lng = load(LNG, [HID, 1])
        lnb = load(LNB, [HID, 1])
        eye48 = load(EYE48, [I, I])
        w3c = load(W3C, [HID, NH], act_dt)
        w3diag = []
        for h in range(NH):
            t = singles.tile([HID, I, I], act_dt, name=f"w3diag{h}",
                             tag=f"w3diag{h}")
            nc.gpsimd.memset(t, 0.0)
            diag_view = bass.AP(tensor=t.tensor, offset=t.offset,
                                ap=[t.ap[0], [I + 1, I]])
            nc.vector.tensor_scalar(out=diag_view,
                                    in0=bcast_free(w3c[:, h:h + 1], I),
                                    scalar1=1.0, scalar2=None, op0=OP.mult)
            w3diag.append(t)

        ones31 = singles.tile([3, 1], f32)
        nc.vector.memset(ones31, 1.0)
        epsc = singles.tile([I, 1], f32)
        nc.vector.memset(epsc, 1e-5)

        # persistent intermediates
        base_sb = singles.tile([HID, I * E], base_dt, tag="base")
        kp1_sb = singles.tile([HID, NH, E], bf16, tag="kp1")
        qpb_sb = singles.tile([HID, NH, I], f32, tag="qpb")
        khead = [singles.tile([DH, E], mm_f32, name=f"khead{h}", tag=f"khead{h}") for h in range(NH)]
        v_ed = [singles.tile([HID, HID], f32, name=f"ved{c}", tag=f"ved{c}") for c in range(3)]
        gateT = singles.tile([I, NH], f32, tag="gateT")
        db_sb = singles.tile([I, E], f32, tag="db")
        f_sb = singles.tile([I, HID], f32, tag="fsb")
        cacc = singles.tile([I, 3], f32, tag="cacc")
        nc.vector.memset(cacc, 0.0)

        # ---------------- phase 0: projections & geometry ----------------
        with tc.tile_pool(name="p0psum", bufs=6, space="PSUM") as pp0, \
             tc.tile_pool(name="p0sb", bufs=4) as sp0:

            # v (all edges, natural layout): v_ed[c] = (X @ wv) rows chunk c
            for c in range(3):
                ps = pp0.tile([HID, HID], f32, tag="ps0")
                nc.tensor.matmul(ps, lhsT=xt[:, c * 128:(c + 1) * 128], rhs=wv)
                nc.vector.tensor_copy(out=v_ed[c], in_=ps)

            # per-head k rows:  khead[h] = (X @ wk)^T rows [16h:16h+16] = wk[:,h]^T X^T
            for h in range(NH):
                ps = pp0.tile([DH, E], f32, tag="ps0")
                nc.tensor.matmul(ps, lhsT=wk[:, h * DH:(h + 1) * DH], rhs=xt)
                nc.vector.tensor_copy(out=khead[h], in_=ps)
                # kp1 = W1k^T khead  [HID, E]
                ps2 = pp0.tile([HID, E], f32, tag="ps0")
                nc.tensor.matmul(ps2, lhsT=w1k, rhs=khead[h])
                nc.vector.tensor_copy(out=kp1_sb[:, h, :], in_=ps2)

            # per-head q rows + qpb = W1q^T qhead + b1
            for h in range(NH):
                ps = pp0.tile([DH, I], f32, tag="ps0")
                nc.tensor.matmul(ps, lhsT=wq[:, h * DH:(h + 1) * DH], rhs=xst)
                qh = sp0.tile([DH, I], mm_f32, tag="qh")
                nc.vector.tensor_copy(out=qh, in_=ps)
                ps2 = pp0.tile([HID, I], f32, tag="ps0")
                nc.tensor.matmul(ps2, lhsT=w1q, rhs=qh)
                nc.vector.tensor_scalar(out=qpb_sb[:, h, :], in0=ps2,
                                        scalar1=b1c, scalar2=None, op0=OP.add)

            # gates: sigmoid(silu(vh gw1 + gb1) gw2 + gb2), slab rows only.
            # All heads share one PSUM tile so each activation runs once.
            g1ps = pp0.tile([HID, NH, 64], f32, tag="g1ps", bufs=1)
            for h in range(NH):
                ps = pp0.tile([DH, I], f32, tag="ps0")
                nc.tensor.matmul(ps, lhsT=wv[:, h * DH:(h + 1) * DH], rhs=xst)
                vh = sp0.tile([DH, I], f32, tag="vh")
                nc.vector.tensor_copy(out=vh, in_=ps)
                nc.tensor.matmul(g1ps[:, h, 0:I], lhsT=gw1, rhs=vh,
                                 skip_group_check=True)
            g1all = sp0.tile([HID, NH, I], f32, tag="g1all")
            nc.scalar.activation(out=g1all, in_=g1ps[:, :, 0:I], func=AF.Silu,
                                 bias=gb1)
            g2ps = pp0.tile([1, NH, 64], f32, tag="g2ps", bufs=1)
            for h in range(NH):
                nc.tensor.matmul(g2ps[0:1, h, 0:I], lhsT=gw2, rhs=g1all[:, h, :],
                                 skip_group_check=True)
            gate_all = singles.tile([1, NH, I], f32, tag="gate_all")
            nc.scalar.activation(out=gate_all, in_=g2ps[:, :, 0:I],
                                 func=AF.Sigmoid, bias=gb2)
            for h in range(NH):
                psg = pp0.tile([I, 1], f32, tag="ps0")
                nc.tensor.transpose(psg, gate_all[0:1, h, :], one11)
                nc.vector.tensor_scalar(out=gateT[:, h:h + 1], in0=psg,
                                        scalar1=1.0 / NH, scalar2=None,
                                        op0=OP.mult)

            # geometry: D2 = -2 cs^T ct + 1 x n2row + n2slab (via psum accum)
            cs2 = sp0.tile([3, I], f32, tag="cs2")
            nc.vector.tensor_scalar(out=cs2, in0=cs, scalar1=-2.0, scalar2=None,
                                    op0=OP.mult)
            sq = sp0.tile([3, E], f32, tag="sq")
            nc.vector.tensor_mul(out=sq, in0=ct, in1=ct)
            psn = pp0.tile([1, E], f32, tag="ps0")
            nc.tensor.matmul(psn, lhsT=ones31, rhs=sq)
            n2row = sp0.tile([1, E], f32, tag="n2row")
            nc.vector.tensor_copy(out=n2row, in_=psn)
            sqs = sp0.tile([3, I], f32, tag="sqs")
            nc.vector.tensor_mul(out=sqs, in0=cs, in1=cs)
            psn2 = pp0.tile([I, 1], f32, tag="ps0")
            nc.tensor.matmul(psn2, lhsT=sqs, rhs=ones31)
            n2slab = sp0.tile([I, 1], f32, tag="n2slab")
            nc.vector.tensor_copy(out=n2slab, in_=psn2)
            ones1x48 = sp0.tile([1, I], f32, tag="ones1x48")
            nc.vector.memset(ones1x48, 1.0)

            psd2 = pp0.tile([I, E], f32, tag="ps0")
            nc.tensor.matmul(psd2, lhsT=cs2, rhs=ct, start=True, stop=False,
                             skip_group_check=True)
            nc.tensor.matmul(psd2, lhsT=ones1x48, rhs=n2row, start=False,
                             stop=True, skip_group_check=True)
            ds2 = sp0.tile([I, E], f32, tag="ds2")
            nc.vector.tensor_scalar(out=ds2, in0=psd2, scalar1=n2slab,
                                    scalar2=0.0, op0=OP.add, op1=OP.max)
            dd = sp0.tile([I, E], f32, tag="dd")
            nc.scalar.activation(out=dd, in_=ds2, func=AF.Sqrt)
            g9 = sp0.tile([I, E], f32, tag="g9")
            nc.vector.tensor_scalar(out=g9, in0=ds2, scalar1=CUTOFF * CUTOFF,
                                    scalar2=1e9, op0=OP.is_gt, op1=OP.mult)
            nc.vector.tensor_add(out=db_sb, in0=dd, in1=g9)
            nc.sync.dma_start(out=DB_DRAM[:], in_=db_sb)

        # ---------------- phase 1: base = W1rd^T [rbf; dot] ----------------
        with tc.tile_pool(name="p1psum", bufs=3, space="PSUM") as pp1, \
             tc.tile_pool(name="p1sb", bufs=4) as sp1:
            blk_sizes = [6, 6, 12, 12, 12]
            i0b = 0
            for blk, bsz in enumerate(blk_sizes):
                g4full = sp1.tile([NR, B1, E], mm_f32, name="g4full", tag="g4")
                g4 = g4full[:, 0:bsz, :]
                nc.sync.dma_start(
                    out=g4.bitcast(f32),
                    in_=DB_DRAM[i0b:i0b + bsz, :].partition_broadcast(NR))
                # (d - c)^2 then exp(-w * .), both on ScalarE (idle at ramp)
                nc.scalar.activation(out=g4, in_=g4, func=AF.Square,
                                     bias=negc)
                nc.scalar.activation(out=g4, in_=g4, func=AF.Exp, scale=negw)
                for u in range(bsz):
                    i = i0b + u
                    lhsd = sp1.tile([3, HID], mm_f32, tag="lhsd")
                    nc.vector.tensor_scalar(out=lhsd, in0=w1d3,
                                            scalar1=cs[:, i:i + 1], scalar2=None,
                                            op0=OP.mult)
                    ps = pp1.tile([HID, E], f32, tag="baseps")
                    nc.tensor.matmul(ps, lhsT=w1r, rhs=g4[:, u, :],
                                     start=True, stop=False, skip_group_check=True)
                    nc.tensor.matmul(ps, lhsT=lhsd, rhs=ct3r,
                                     start=False, stop=True, skip_group_check=True)
                    nc.any.tensor_copy(out=base_sb[:, i * E:(i + 1) * E], in_=ps)
                i0b += bsz

        # ---------------- phase 2: attention heads ----------------
        B6 = 6
        with tc.tile_pool(name="z2psum", bufs=1, space="PSUM") as zp, \
             tc.tile_pool(name="spsum", bufs=1, space="PSUM") as sp_pool, \
             tc.tile_pool(name="mpsum", bufs=1, space="PSUM") as mp, \
             tc.tile_pool(name="p2sb", bufs=4) as sb2:

            GH = 2
            pm = singles.tile([I, GH, E], f32, tag="pm")
            pm_done = []
            for h in range(NH):
                s_ps = sp_pool.tile([I, E], f32, tag="S")
                for blk in range(I // B6):
                    i0 = blk * B6
                    z1 = sb2.tile([HID, B6, E], bf16, tag="z1", bufs=2)
                    nc.vector.tensor_add(
                        out=z1,
                        in0=base_sb[:, i0 * E:(i0 + B6) * E].rearrange(
                            "p (u e) -> p u e", u=B6),
                        in1=bcast_free(kp1_sb[:, h, :], B6))
                    for u in range(B6):
                        i = i0 + u
                        nc.vector.tensor_scalar(out=z1[:, u, :], in0=z1[:, u, :],
                                                scalar1=qpb_sb[:, h, i:i + 1],
                                                scalar2=None, op0=OP.add)
                    x1 = sb2.tile([HID, B6, E], x1_dt, tag="x1", bufs=2)
                    nc.scalar.activation(out=x1, in_=z1, func=AF.Silu)
                    for half in range(B6 // B2):
                        z2 = zp.tile([HID, B2, 512], f32, tag="z2")
                        for u3 in range(B2):
                            u = half * B2 + u3
                            nc.tensor.matmul(z2[:, u3, 0:E], lhsT=w2,
                                             rhs=x1[:, u, :])
                        x2 = sb2.tile([HID, B2, E], act_dt, tag="x2")
                        nc.scalar.activation(out=x2, in_=z2[:, :, 0:E],
                                             func=AF.Silu, bias=b2c)
                        for u3 in range(B2):
                            i = i0 + half * B2 + u3
                            nc.tensor.matmul(
                                s_ps, lhsT=w3diag[h][:, i, :], rhs=x2[:, u3, :],
                                start=(i == 0), stop=False, skip_group_check=True)
                # + mask
                nc.tensor.matmul(s_ps, lhsT=eye48r, rhs=mask, start=False,
                                 stop=True, skip_group_check=True)
                negmax = sb2.tile([I, 1], f32, tag="negmax")
                nc.vector.reduce_max(negmax, s_ps, axis=AX.X, negate=True)
                nc.vector.tensor_scalar(out=pm[:, h % GH, :], in0=s_ps,
                                        scalar1=negmax, scalar2=None, op0=OP.add)
                if h % GH == GH - 1:
                    pms = sb2.tile([I, GH, E], f32, name=f"pms{h}", tag="pms")
                    nc.scalar.activation(out=pms, in_=pm, func=AF.Exp)
                    pm_done.append(pms)

            # ------------- phase 2b: softmax + attn application -------------
            for h in range(NH):
                p_sb = pm_done[h // GH][:, h % GH, :]
                rs = sb2.tile([I, 1], f32, tag="rs")
                nc.vector.reduce_sum(rs, p_sb, axis=AX.X)
                rinv = sb2.tile([I, 1], f32, tag="rinv")
                nc.vector.reciprocal(rinv, rs)
                a_sb = sb2.tile([I, E], f32, tag="a")
                nc.vector.tensor_scalar(out=a_sb, in0=p_sb, scalar1=rinv,
                                        scalar2=None, op0=OP.mult)

                at = [sb2.tile([128, I], f32, name=f"at{c}", tag=f"at{c}")
                      for c in range(3)]
                for c in range(3):
                    pst = mp.tile([128, I], f32, tag="mps")
                    nc.tensor.transpose(pst, a_sb[:, c * 128:(c + 1) * 128], eye48)
                    nc.vector.tensor_copy(out=at[c], in_=pst)
                fps = mp.tile([I, DH], f32, tag="mps")
                for c in range(3):
                    nc.tensor.matmul(fps, lhsT=at[c],
                                     rhs=v_ed[c][:, h * DH:(h + 1) * DH],
                                     start=(c == 0), stop=(c == 2))
                nc.vector.tensor_copy(out=f_sb[:, h * DH:(h + 1) * DH], in_=fps)
                cps = mp.tile([I, 3], f32, tag="mps")
                for c in range(3):
                    nc.tensor.matmul(cps, lhsT=at[c], rhs=ce[c],
                                     start=(c == 0), stop=(c == 2))
                cw = sb2.tile([I, 3], f32, tag="cw")
                nc.vector.tensor_scalar(out=cw, in0=cps,
                                        scalar1=gateT[:, h:h + 1], scalar2=None,
                                        op0=OP.mult)
                nc.vector.tensor_add(out=cacc, in0=cacc, in1=cw)

            # ---------------- epilogue ----------------
            ftps = mp.tile([HID, I], f32, tag="mps")
            nc.tensor.transpose(ftps, f_sb, eye48)
            ft = sb2.tile([HID, I], f32, tag="ft")
            nc.vector.tensor_copy(out=ft, in_=ftps)
            yps = mp.tile([I, HID], f32, tag="mps")
            nc.tensor.matmul(yps, lhsT=ft, rhs=wo, start=True, stop=False,
                             skip_group_check=True)
            nc.tensor.matmul(yps, lhsT=eye48, rhs=fs, start=False, stop=False,
                             skip_group_check=True)
            ones1I = sb2.tile([1, I], f32, tag="ones1I")
            nc.vector.memset(ones1I, 1.0)
            nc.tensor.matmul(yps, lhsT=ones1I, rhs=bor, start=False, stop=True,
                             skip_group_check=True)

            stats = sb2.tile([I, 6], f32, tag="stats")
            nc.vector.bn_stats(out=stats, in_=yps)
            mv = sb2.tile([I, 2], f32, tag="mv")
            nc.vector.bn_aggr(out=mv, in_=stats)
            sd = sb2.tile([I, 1], f32, tag="sd")
            nc.scalar.activation(out=sd, in_=mv[:, 1:2], func=AF.Sqrt,
                                 bias=epsc)
            rstd = sb2.tile([I, 1], f32, tag="rstd")
            nc.vector.reciprocal(rstd, sd)
            yc = sb2.tile([I, HID], f32, tag="yc")
            nc.vector.tensor_scalar(out=yc, in0=yps, scalar1=mv[:, 0:1],
                                    scalar2=rstd, op0=OP.subtract, op1=OP.mult)
            ytps = mp.tile([HID, I], f32, tag="mps")
            nc.tensor.transpose(ytps, yc, eye48)
            outt = sb2.tile([HID, I], f32, tag="outt")
            nc.vector.tensor_scalar(out=outt, in0=ytps, scalar1=lng, scalar2=lnb,
                                    op0=OP.mult, op1=OP.add)
            nc.sync.dma_start(out=OUTFT[:], in_=outt)

            # coords_out = csl + sumg * csl_xi - cacc  (sumg = sum_h gate/NH)
            sumg = sb2.tile([I, 1], f32, tag="sumg")
            nc.vector.reduce_sum(sumg, gateT, axis=AX.X)
            own = sb2.tile([I, 3], f32, tag="own")
            nc.vector.tensor_scalar(out=own, in0=csl, scalar1=sumg, scalar2=None,
                                    op0=OP.mult)
            co = sb2.tile([I, 3], f32, tag="co")
            nc.vector.tensor_add(out=co, in0=csl, in1=own)
            nc.vector.tensor_sub(out=co, in0=co, in1=cacc)
            nc.sync.dma_start(out=OUTC[:], in_=co)

    _split_excess_waits(nc)
    return nc


_CACHED = {}


def kernel(**inputs):
    import ml_dtypes

    bf16np = ml_dtypes.bfloat16

    feat = np.asarray(inputs["edge_features"], np.float32)
    coords = np.asarray(inputs["edge_coords"], np.float32)
    mask = np.asarray(inputs["edge_mask"], np.float32)
    wq = np.asarray(inputs["wq"], np.float32)
    wk = np.asarray(inputs["wk"], np.float32)
    wv = np.asarray(inputs["wv"], np.float32)
    centers = np.asarray(inputs["rbf_centers"], np.float32)
    widths = np.asarray(inputs["rbf_widths"], np.float32)
    a_w1 = np.asarray(inputs["a_w1"], np.float32)
    a_b1 = np.asarray(inputs["a_b1"], np.float32)
    a_w2 = np.asarray(inputs["a_w2"], np.float32)
    a_b2 = np.asarray(inputs["a_b2"], np.float32)
    a_w3 = np.asarray(inputs["a_w3"], np.float32)
    g_w1 = np.asarray(inputs["g_w1"], np.float32)
    g_b1 = np.asarray(inputs["g_b1"], np.float32)
    g_w2 = np.asarray(inputs["g_w2"], np.float32)
    g_b2 = np.asarray(inputs["g_b2"], np.float32)
    wo = np.asarray(inputs["wo"], np.float32)
    bo = np.asarray(inputs["bo"], np.float32)
    ln_g = np.asarray(inputs["ln_g"], np.float32)
    ln_b = np.asarray(inputs["ln_b"], np.float32)

    if "nc" not in _CACHED:
        _CACHED["nc"] = build_program()
    nc = _CACHED["nc"]

    eye = np.eye(I, dtype=np.float32)

    rep = {
        "XT": np.ascontiguousarray(feat.T),
        "CT": np.ascontiguousarray(coords.T),
        "CE": coords,
        "WQ": wq, "WK": wk, "WV": wv,
        "W1Q": np.ascontiguousarray(a_w1[0:DH]),
        "W1K": np.ascontiguousarray(a_w1[DH:2 * DH]),
        "W1R": np.ascontiguousarray(a_w1[2 * DH:2 * DH + NR]),
        "W1D3": np.tile(a_w1[2 * DH + NR:2 * DH + NR + 1], (3, 1)),
        "CT3R": np.ascontiguousarray(coords.T),
        "B1C": a_b1.reshape(HID, 1),
        "W2": a_w2.astype(np.float32 if X1_FP32 else bf16np),
        "B2C": a_b2.reshape(HID, 1),
        "W3C": a_w3.astype(bf16np),
        "NEGC": -(centers - 1e-8).reshape(NR, 1),
        "NEGW": (-widths).reshape(NR, 1),
        "GW1": g_w1, "GB1": g_b1.reshape(HID, 1),
        "GW2": g_w2, "GB2": g_b2.reshape(1, 1),
        "WO": wo, "BOR": bo.reshape(1, HID),
        "LNG": ln_g.reshape(HID, 1), "LNB": ln_b.reshape(HID, 1),
        "EYE48": eye, "EYE48R": eye,
    }
    in_maps = []
    for c in range(NC):
        sl = slice(c * I, (c + 1) * I)
        m = dict(rep)
        m["XST"] = np.ascontiguousarray(feat[sl].T)
        m["FS"] = feat[sl]
        m["CS"] = np.ascontiguousarray(coords[sl].T)
        m["CSL"] = coords[sl]
        m["MASK"] = mask[sl]
        in_maps.append(m)

    from concourse.bass_utils import run_bass_kernel_spmd

    res = run_bass_kernel_spmd(nc, in_maps, list(range(NC)))
    out_f = np.concatenate([res.results[c]["OUTFT"].T for c in range(NC)], axis=0)
    coords_out = np.concatenate([res.results[c]["OUTC"] for c in range(NC)], axis=0)
    return out_f.astype(np.float32), coords_out.astype(np.float32)


# revision 52
# speedup vs baseline: 1.2566x; 1.2566x over previous
"""Trainium2 Bass kernel for EquivariantEdgeAttention (E=384, HID=128, NH=8).

Sharding: 8 cores, core c computes query-edge rows [48c, 48c+48) of the
pairwise attention; params / keys / values / coords are replicated.
Outputs are gathered by concatenation on the host. No collectives.

Self-contained: only numpy + concourse (bass) imports, no sibling files.
"""

import numpy as np

E = 384
NC = 8
I = E // NC          # 48 query rows per core
HID = 128
NH = 8
DH = 16
NR = 64
CUTOFF = 10.0

# ---- tuning config ----
B2 = 3               # i-block size in phase 2 (z2 psum banks per block)
B1 = 12              # i-block size in phase 1 (rbf/base build)
ACT_DT = None        # filled below (bf16) - dtype of x2 / W3diag path
BASE_BF16 = True     # store base (rbf+dot proj) as bf16
X1_FP32 = True       # x1 (L2 rhs) and W2 in fp32 (vs bf16)
MM_F32R = True      # use float32r for big fp32 matmuls


# --------------------------------------------------------------------------
# walrus workaround: this container's walrus rejects >1 sync wait on a CTRL
# Drain.  Split the TileContext tail drain into single-wait drains.
# --------------------------------------------------------------------------
def _patch_tile_drain():
    import concourse.tile as tile
    from concourse import mybir
    from concourse.vector_clock import ScopedClock

    def _drain_and_barrier_split(self, tick_clock, wait_clock):
        nc = self.nc
        drain_inst = nc.sync.drain()
        wait_clock.add_sem_waits(
            drain_inst.ins, ScopedClock({None: tick_clock.global_clock})
        )
        si = drain_inst.ins.sync_info
        waits = list(si.on_wait or [])
        if len(waits) > 1:
            drain_inst.ins.sync_info = mybir.SyncInfo(
                on_wait=waits[:1], on_update=list(si.on_update or [])
            )
            for w in waits[1:]:
                d2 = nc.sync.drain()
                d2.ins.sync_info = mybir.SyncInfo(on_wait=[w], on_update=[])
        nc.all_engine_barrier()
        assert self.sems is not None
        popped = nc._tile_sem_poison_stack.pop()
        assert popped is self._sem_poison
        nc.clear_and_free_semaphores(list(self.sems.allocated().values()))
        nc.all_engine_barrier()

    tile.TileContext._drain_and_barrier = _drain_and_barrier_split


def _split_excess_waits(nc, max_waits=1):
    """This container's walrus supports only one sync-wait command per
    engine instruction.  Move excess waits onto single-wait NoOps inserted
    immediately before the instruction on the same engine.  DMA-class
    instructions keep their waits (queue-mediated, not engine-decoded)."""
    from concourse import mybir

    n = 0
    for f in nc.m.functions:
        for bb in f.blocks:
            out = []
            for inst in bb.instructions:
                tn = type(inst).__name__
                si = inst.sync_info
                waits = list(si.on_wait) if si and si.on_wait else []
                if len(waits) > max_waits:
                    for w in waits[:-max_waits]:
                        nop = mybir.InstNoOp(
                            name=f"{inst.name}-w{n}", ins=[], outs=[],
                            engine=inst.engine)
                        nop.sync_info = mybir.SyncInfo(on_wait=[w], on_update=[])
                        out.append(nop)
                        n += 1
                    inst.sync_info = mybir.SyncInfo(
                        on_wait=waits[-max_waits:],
                        on_update=list(si.on_update or []))
                out.append(inst)
            bb.instructions = out
    return n


def build_program():
    import concourse.bass as bass
    import concourse.tile as tile
    from concourse import mybir
    from contextlib import ExitStack

    _patch_tile_drain()

    f32 = mybir.dt.float32
    bf16 = mybir.dt.bfloat16
    AF = mybir.ActivationFunctionType
    OP = mybir.AluOpType
    AX = mybir.AxisListType

    act_dt = bf16                      # x2 / W3diag dtype
    f32r = mybir.dt.float32r
    mm_f32 = f32r if MM_F32R else f32   # dtype of fp32 tensors feeding matmuls
    x1_dt = mm_f32 if X1_FP32 else bf16   # x1 / W2 dtype
    base_dt = bf16 if BASE_BF16 else f32

    nc = bass.Bass("TRN2", num_devices=NC)

    def din(name, shape, dt=f32):
        return nc.declare_dram_parameter(name, list(shape), dt, isOutput=False)

    # ---------------- dram inputs ----------------
    XT = din("XT", [HID, E], mm_f32)       # features^T (replicated)
    XST = din("XST", [HID, I], mm_f32)     # slab features^T
    FS = din("FS", [I, HID])              # slab features (residual)
    CT = din("CT", [3, E])                # coords^T
    CS = din("CS", [3, I])                # slab coords^T
    CE = din("CE", [E, 3])                # coords natural
    CSL = din("CSL", [I, 3])              # slab coords natural
    MASK = din("MASK", [I, E], mm_f32)
    EYE48R = din("EYE48R", [I, I], mm_f32)
    WQ = din("WQ", [HID, HID], mm_f32)
    WK = din("WK", [HID, HID], mm_f32)
    WV = din("WV", [HID, HID], mm_f32)
    W1Q = din("W1Q", [DH, HID], mm_f32)
    W1K = din("W1K", [DH, HID], mm_f32)
    W1R = din("W1R", [NR, HID], mm_f32)   # a_w1 rbf rows
    W1D3 = din("W1D3", [3, HID])          # a_w1 dot row replicated x3
    CT3R = din("CT3R", [3, E], mm_f32)    # coords^T for dot-term matmul
    B1C = din("B1C", [HID, 1])
    W2 = din("W2", [HID, HID], x1_dt)
    B2C = din("B2C", [HID, 1])
    W3C = din("W3C", [HID, NH], act_dt)   # a_w3
    NEGC = din("NEGC", [NR, 1])           # -(centers - 1e-8)
    NEGW = din("NEGW", [NR, 1])           # -widths
    GW1 = din("GW1", [DH, HID])
    GB1 = din("GB1", [HID, 1])
    GW2 = din("GW2", [HID, 1])
    GB2 = din("GB2", [1, 1])
    WO = din("WO", [HID, HID])
    BOR = din("BOR", [1, HID])
    LNG = din("LNG", [HID, 1])
    LNB = din("LNB", [HID, 1])
    EYE48 = din("EYE48", [I, I])

    OUTFT = nc.declare_dram_parameter("OUTFT", [HID, I], f32, isOutput=True)
    OUTC = nc.declare_dram_parameter("OUTC", [I, 3], f32, isOutput=True)

    DB_DRAM = nc.dram_tensor("db_scratch", [I, E], f32)

    def bcast_free(ap, n):
        """Insert a stride-0 dim of size n after the partition dim: [P, F] -> [P, n, F]."""
        return bass.AP(tensor=ap.tensor, offset=ap.offset,
                       ap=[ap.ap[0], [0, n]] + list(ap.ap[1:]))

    with tile.TileContext(nc) as tc, ExitStack() as ctx:
        singles = ctx.enter_context(tc.tile_pool(name="singles", bufs=1))

        def load(dram, shape, dt=f32, name=None):
            t = singles.tile(list(shape), dt, tag=name or dram.name)
            nc.sync.dma_start(out=t, in_=dram[:])
            return t

        # ---------------- load constants ----------------
        xt = load(XT, [HID, E], mm_f32)
        xst = load(XST, [HID, I], mm_f32)
        fs = load(FS, [I, HID])
        ct = load(CT, [3, E])
        cs = load(CS, [3, I])
        ce = []
        for c in range(3):
            t = singles.tile([128, 3], f32, tag=f"ce{c}")
            nc.sync.dma_start(out=t, in_=CE[c * 128:(c + 1) * 128, :])
            ce.append(t)
        csl = load(CSL, [I, 3])
        mask = load(MASK, [I, E], mm_f32)
        eye48r = load(EYE48R, [I, I], mm_f32)
        wq = load(WQ, [HID, HID], mm_f32)
        wk = load(WK, [HID, HID], mm_f32)
        wv = load(WV, [HID, HID], mm_f32)
        w1q = load(W1Q, [DH, HID], mm_f32)
        w1k = load(W1K, [DH, HID], mm_f32)
        w1r = load(W1R, [NR, HID], mm_f32)
        w1d3 = load(W1D3, [3, HID])
        ct3r = load(CT3R, [3, E], mm_f32)
        b1c = load(B1C, [HID, 1])
        w2 = load(W2, [HID, HID], x1_dt)
        b2c = load(B2C, [HID, 1])
        negc = load(NEGC, [NR, 1])
        negw = load(NEGW, [NR, 1])
        gw1 = load(GW1, [DH, HID])
        gb1 = load(GB1, [HID, 1])
        gw2 = load(GW2, [HID, 1])
        gb2 = load(GB2, [1, 1])
        one11 = singles.tile([1, 1], f32, tag="one11id")
        nc.vector.memset(one11, 1.0)
        wo = load(WO, [HID, HID])
        bor = load(BOR, [1, HID])
        lng = load(LNG, [HID, 1])
        lnb = load(LNB, [HID, 1])
        eye48 = load(EYE48, [I, I])
        w3c = load(W3C, [HID, NH], act_dt)
        w3diag = []
        for h in range(NH):
            t = singles.tile([HID, I, I], act_dt, name=f"w3diag{h}",
                             tag=f"w3diag{h}")
            nc.gpsimd.memset(t, 0.0)
            diag_view = bass.AP(tensor=t.tensor, offset=t.offset,
                                ap=[t.ap[0], [I + 1, I]])
            nc.vector.tensor_scalar(out=diag_view,
                                    in0=bcast_free(w3c[:, h:h + 1], I),
                                    scalar1=1.0, scalar2=None, op0=OP.mult)
            w3diag.append(t)

        ones31 = singles.tile([3, 1], f32)
        nc.vector.memset(ones31, 1.0)
        epsc = singles.tile([I, 1], f32)
        nc.vector.memset(epsc, 1e-5)

        # persistent intermediates
        base_sb = singles.tile([HID, I * E], base_dt, tag="base")
        kp1_sb = singles.tile([HID, NH, E], bf16, tag="kp1")
        qpb_sb = singles.tile([HID, NH, I], f32, tag="qpb")
        khead = [singles.tile([DH, E], mm_f32, name=f"khead{h}", tag=f"khead{h}") for h in range(NH)]
        v_ed = [singles.tile([HID, HID], f32, name=f"ved{c}", tag=f"ved{c}") for c in range(3)]
        gateT = singles.tile([I, NH], f32, tag="gateT")
        db_sb = singles.tile([I, E], f32, tag="db")
        f_sb = singles.tile([I, HID], f32, tag="fsb")
        cacc = singles.tile([I, 3], f32, tag="cacc")
        nc.vector.memset(cacc, 0.0)

        # ---------------- phase 0: projections & geometry ----------------
        with tc.tile_pool(name="p0psum", bufs=6, space="PSUM") as pp0, \
             tc.tile_pool(name="p0sb", bufs=4) as sp0:

            # v (all edges, natural layout): v_ed[c] = (X @ wv) rows chunk c
            for c in range(3):
                ps = pp0.tile([HID, HID], f32, tag="ps0")
                nc.tensor.matmul(ps, lhsT=xt[:, c * 128:(c + 1) * 128], rhs=wv)
                nc.vector.tensor_copy(out=v_ed[c], in_=ps)

            # per-head k rows:  khead[h] = (X @ wk)^T rows [16h:16h+16] = wk[:,h]^T X^T
            for h in range(NH):
                ps = pp0.tile([DH, E], f32, tag="ps0")
                nc.tensor.matmul(ps, lhsT=wk[:, h * DH:(h + 1) * DH], rhs=xt)
                nc.vector.tensor_copy(out=khead[h], in_=ps)
                # kp1 = W1k^T khead  [HID, E]
                ps2 = pp0.tile([HID, E], f32, tag="ps0")
                nc.tensor.matmul(ps2, lhsT=w1k, rhs=khead[h])
                nc.vector.tensor_copy(out=kp1_sb[:, h, :], in_=ps2)

            # per-head q rows + qpb = W1q^T qhead + b1
            for h in range(NH):
                ps = pp0.tile([DH, I], f32, tag="ps0")
                nc.tensor.matmul(ps, lhsT=wq[:, h * DH:(h + 1) * DH], rhs=xst)
                qh = sp0.tile([DH, I], mm_f32, tag="qh")
                nc.vector.tensor_copy(out=qh, in_=ps)
                ps2 = pp0.tile([HID, I], f32, tag="ps0")
                nc.tensor.matmul(ps2, lhsT=w1q, rhs=qh)
                nc.vector.tensor_scalar(out=qpb_sb[:, h, :], in0=ps2,
                                        scalar1=b1c, scalar2=None, op0=OP.add)

            # gates: sigmoid(silu(vh gw1 + gb1) gw2 + gb2), slab rows only.
            # All heads share one PSUM tile so each activation runs once.
            g1ps = pp0.tile([HID, NH, 64], f32, tag="g1ps", bufs=1)
            for h in range(NH):
                ps = pp0.tile([DH, I], f32, tag="ps0")
                nc.tensor.matmul(ps, lhsT=wv[:, h * DH:(h + 1) * DH], rhs=xst)
                vh = sp0.tile([DH, I], f32, tag="vh")
                nc.vector.tensor_copy(out=vh, in_=ps)
                nc.tensor.matmul(g1ps[:, h, 0:I], lhsT=gw1, rhs=vh,
                                 skip_group_check=True)
            g1all = sp0.tile([HID, NH, I], f32, tag="g1all")
            nc.scalar.activation(out=g1all, in_=g1ps[:, :, 0:I], func=AF.Silu,
                                 bias=gb1)
            g2ps = pp0.tile([1, NH, 64], f32, tag="g2ps", bufs=1)
            for h in range(NH):
                nc.tensor.matmul(g2ps[0:1, h, 0:I], lhsT=gw2, rhs=g1all[:, h, :],
                                 skip_group_check=True)
            gate_all = singles.tile([1, NH, I], f32, tag="gate_all")
            nc.scalar.activation(out=gate_all, in_=g2ps[:, :, 0:I],
                                 func=AF.Sigmoid, bias=gb2)
            for h in range(NH):
                psg = pp0.tile([I, 1], f32, tag="ps0")
                nc.tensor.transpose(psg, gate_all[0:1, h, :], one11)
                nc.vector.tensor_scalar(out=gateT[:, h:h + 1], in0=psg,
                                        scalar1=1.0 / NH, scalar2=None,
                                        op0=OP.mult)

            # geometry: D2 = -2 cs^T ct + 1 x n2row + n2slab (via psum accum)
            cs2 = sp0.tile([3, I], f32, tag="cs2")
            nc.vector.tensor_scalar(out=cs2, in0=cs, scalar1=-2.0, scalar2=None,
                                    op0=OP.mult)
            sq = sp0.tile([3, E], f32, tag="sq")
            nc.vector.tensor_mul(out=sq, in0=ct, in1=ct)
            psn = pp0.tile([1, E], f32, tag="ps0")
            nc.tensor.matmul(psn, lhsT=ones31, rhs=sq)
            n2row = sp0.tile([1, E], f32, tag="n2row")
            nc.vector.tensor_copy(out=n2row, in_=psn)
            sqs = sp0.tile([3, I], f32, tag="sqs")
            nc.vector.tensor_mul(out=sqs, in0=cs, in1=cs)
            psn2 = pp0.tile([I, 1], f32, tag="ps0")
            nc.tensor.matmul(psn2, lhsT=sqs, rhs=ones31)
            n2slab = sp0.tile([I, 1], f32, tag="n2slab")
            nc.vector.tensor_copy(out=n2slab, in_=psn2)
            ones1x48 = sp0.tile([1, I], f32, tag="ones1x48")
            nc.vector.memset(ones1x48, 1.0)

            psd2 = pp0.tile([I, E], f32, tag="ps0")
            nc.tensor.matmul(psd2, lhsT=cs2, rhs=ct, start=True, stop=False,
                             skip_group_check=True)
            nc.tensor.matmul(psd2, lhsT=ones1x48, rhs=n2row, start=False,
                             stop=True, skip_group_check=True)
            ds2 = sp0.tile([I, E], f32, tag="ds2")
            nc.vector.tensor_scalar(out=ds2, in0=psd2, scalar1=n2slab,
                                    scalar2=0.0, op0=OP.add, op1=OP.max)
            dd = sp0.tile([I, E], f32, tag="dd")
            nc.scalar.activation(out=dd, in_=ds2, func=AF.Sqrt)
            g9 = sp0.tile([I, E], f32, tag="g9")
            nc.vector.tensor_scalar(out=g9, in0=ds2, scalar1=CUTOFF * CUTOFF,
                                    scalar2=1e9, op0=OP.is_gt, op1=OP.mult)
            nc.vector.tensor_add(out=db_sb, in0=dd, in1=g9)
            nc.sync.dma_start(out=DB_DRAM[:], in_=db_sb)

        # ---------------- phase 1: base = W1rd^T [rbf; dot] ----------------
        with tc.tile_pool(name="p1psum", bufs=3, space="PSUM") as pp1, \
             tc.tile_pool(name="p1sb", bufs=4) as sp1:
            blk_sizes = [6, 6, 12, 12, 12]
            i0b = 0
            for blk, bsz in enumerate(blk_sizes):
                g4full = sp1.tile([NR, B1, E], mm_f32, name="g4full", tag="g4")
                g4 = g4full[:, 0:bsz, :]
                nc.sync.dma_start(
                    out=g4.bitcast(f32),
                    in_=DB_DRAM[i0b:i0b + bsz, :].partition_broadcast(NR))
                # (d - c)^2 then exp(-w * .), both on ScalarE (idle at ramp)
                nc.scalar.activation(out=g4, in_=g4, func=AF.Square,
                                     bias=negc)
                nc.scalar.activation(out=g4, in_=g4, func=AF.Exp, scale=negw)
                for u in range(bsz):
                    i = i0b + u
                    lhsd = sp1.tile([3, HID], mm_f32, tag="lhsd")
                    nc.vector.tensor_scalar(out=lhsd, in0=w1d3,
                                            scalar1=cs[:, i:i + 1], scalar2=None,
                                            op0=OP.mult)
                    ps = pp1.tile([HID, E], f32, tag="baseps")
                    nc.tensor.matmul(ps, lhsT=w1r, rhs=g4[:, u, :],
                                     start=True, stop=False, skip_group_check=True)
                    nc.tensor.matmul(ps, lhsT=lhsd, rhs=ct3r,
                                     start=False, stop=True, skip_group_check=True)
                    nc.any.tensor_copy(out=base_sb[:, i * E:(i + 1) * E], in_=ps)
                i0b += bsz

        # ---------------- phase 2: attention heads ----------------
        B6 = 6
        with tc.tile_pool(name="z2psum", bufs=2, space="PSUM") as zp, \
             tc.tile_pool(name="spsum", bufs=1, space="PSUM") as sp_pool, \
             tc.tile_pool(name="mpsum", bufs=1, space="PSUM") as mp, \
             tc.tile_pool(name="p2sb", bufs=4) as sb2:

            GH = 2
            pm = singles.tile([I, GH, E], f32, tag="pm")
            pm_done = []
            for h in range(NH):
                s_ps = sp_pool.tile([I, E], f32, tag="S")
                for blk in range(I // B6):
                    i0 = blk * B6
                    z1 = sb2.tile([HID, B6, E], bf16, tag="z1", bufs=2)
                    nc.vector.tensor_add(
                        out=z1,
                        in0=base_sb[:, i0 * E:(i0 + B6) * E].rearrange(
                            "p (u e) -> p u e", u=B6),
                        in1=bcast_free(kp1_sb[:, h, :], B6))
                    for u in range(B6):
                        i = i0 + u
                        nc.vector.tensor_scalar(out=z1[:, u, :], in0=z1[:, u, :],
                                                scalar1=qpb_sb[:, h, i:i + 1],
                                                scalar2=None, op0=OP.add)
                    x1 = sb2.tile([HID, B6, E], x1_dt, tag="x1", bufs=2)
                    nc.scalar.activation(out=x1, in_=z1, func=AF.Silu)
                    for half in range(B6 // B2):
                        z2 = zp.tile([HID, B2, 512], f32, tag="z2")
                        for u3 in range(B2):
                            u = half * B2 + u3
                            nc.tensor.matmul(z2[:, u3, 0:E], lhsT=w2,
                                             rhs=x1[:, u, :])
                        x2 = sb2.tile([HID, B2, E], act_dt, tag="x2")
                        nc.scalar.activation(out=x2, in_=z2[:, :, 0:E],
                                             func=AF.Silu, bias=b2c)
                        for u3 in range(B2):
                            i = i0 + half * B2 + u3
                            nc.tensor.matmul(
                                s_ps, lhsT=w3diag[h][:, i, :], rhs=x2[:, u3, :],
                                start=(i == 0), stop=False, skip_group_check=True)
                # + mask
                nc.tensor.matmul(s_ps, lhsT=eye48r, rhs=mask, start=False,
                                 stop=True, skip_group_check=True)
                negmax = sb2.tile([I, 1], f32, tag="negmax")
                nc.vector.reduce_max(negmax, s_ps, axis=AX.X, negate=True)
                nc.vector.tensor_scalar(out=pm[:, h % GH, :], in0=s_ps,
                                        scalar1=negmax, scalar2=None, op0=OP.add)
                if h % GH == GH - 1:
                    pms = sb2.tile([I, GH, E], f32, name=f"pms{h}", tag="pms")
                    nc.scalar.activation(out=pms, in_=pm, func=AF.Exp)
                    pm_done.append(pms)

            # ------------- phase 2b: softmax + attn application -------------
            for h in range(NH):
                p_sb = pm_done[h // GH][:, h % GH, :]
                rs = sb2.tile([I, 1], f32, tag="rs")
                nc.vector.reduce_sum(rs, p_sb, axis=AX.X)
                rinv = sb2.tile([I, 1], f32, tag="rinv")
                nc.vector.reciprocal(rinv, rs)
                a_sb = sb2.tile([I, E], f32, tag="a")
                nc.vector.tensor_scalar(out=a_sb, in0=p_sb, scalar1=rinv,
                                        scalar2=None, op0=OP.mult)

                at = [sb2.tile([128, I], f32, name=f"at{c}", tag=f"at{c}")
                      for c in range(3)]
                for c in range(3):
                    pst = mp.tile([128, I], f32, tag="mps")
                    nc.tensor.transpose(pst, a_sb[:, c * 128:(c + 1) * 128], eye48)
                    nc.vector.tensor_copy(out=at[c], in_=pst)
                fps = mp.tile([I, DH], f32, tag="mps")
                for c in range(3):
                    nc.tensor.matmul(fps, lhsT=at[c],
                                     rhs=v_ed[c][:, h * DH:(h + 1) * DH],
                                     start=(c == 0), stop=(c == 2))
                nc.vector.tensor_copy(out=f_sb[:, h * DH:(h + 1) * DH], in_=fps)
                cps = mp.tile([I, 3], f32, tag="mps")
                for c in range(3):
                    nc.tensor.matmul(cps, lhsT=at[c], rhs=ce[c],
                                     start=(c == 0), stop=(c == 2))
                cw = sb2.tile([I, 3], f32, tag="cw")
                nc.vector.tensor_scalar(out=cw, in0=cps,
                                        scalar1=gateT[:, h:h + 1], scalar2=None,
                                        op0=OP.mult)
                nc.vector.tensor_add(out=cacc, in0=cacc, in1=cw)

            # ---------------- epilogue ----------------
            ftps = mp.tile([HID, I], f32, tag="mps")
            nc.tensor.transpose(ftps, f_sb, eye48)
            ft = sb2.tile([HID, I], f32, tag="ft")
            nc.vector.tensor_copy(out=ft, in_=ftps)
            yps = mp.tile([I, HID], f32, tag="mps")
            nc.tensor.matmul(yps, lhsT=ft, rhs=wo, start=True, stop=False,
                             skip_group_check=True)
            nc.tensor.matmul(yps, lhsT=eye48, rhs=fs, start=False, stop=False,
                             skip_group_check=True)
            ones1I = sb2.tile([1, I], f32, tag="ones1I")
            nc.vector.memset(ones1I, 1.0)
            nc.tensor.matmul(yps, lhsT=ones1I, rhs=bor, start=False, stop=True,
                             skip_group_check=True)

            stats = sb2.tile([I, 6], f32, tag="stats")
            nc.vector.bn_stats(out=stats, in_=yps)
            mv = sb2.tile([I, 2], f32, tag="mv")
            nc.vector.bn_aggr(out=mv, in_=stats)
            sd = sb2.tile([I, 1], f32, tag="sd")
            nc.scalar.activation(out=sd, in_=mv[:, 1:2], func=AF.Sqrt,
                                 bias=epsc)
            rstd = sb2.tile([I, 1], f32, tag="rstd")
            nc.vector.reciprocal(rstd, sd)
            yc = sb2.tile([I, HID], f32, tag="yc")
            nc.vector.tensor_scalar(out=yc, in0=yps, scalar1=mv[:, 0:1],
                                    scalar2=rstd, op0=OP.subtract, op1=OP.mult)
            ytps = mp.tile([HID, I], f32, tag="mps")
            nc.tensor.transpose(ytps, yc, eye48)
            outt = sb2.tile([HID, I], f32, tag="outt")
            nc.vector.tensor_scalar(out=outt, in0=ytps, scalar1=lng, scalar2=lnb,
                                    op0=OP.mult, op1=OP.add)
            nc.sync.dma_start(out=OUTFT[:], in_=outt)

            # coords_out = csl + sumg * csl_xi - cacc  (sumg = sum_h gate/NH)
            sumg = sb2.tile([I, 1], f32, tag="sumg")
            nc.vector.reduce_sum(sumg, gateT, axis=AX.X)
            own = sb2.tile([I, 3], f32, tag="own")
            nc.vector.tensor_scalar(out=own, in0=csl, scalar1=sumg, scalar2=None,
                                    op0=OP.mult)
            co = sb2.tile([I, 3], f32, tag="co")
            nc.vector.tensor_add(out=co, in0=csl, in1=own)
            nc.vector.tensor_sub(out=co, in0=co, in1=cacc)
            nc.sync.dma_start(out=OUTC[:], in_=co)

    _split_excess_waits(nc)
    return nc


_CACHED = {}


def kernel(**inputs):
    import ml_dtypes

    bf16np = ml_dtypes.bfloat16

    feat = np.asarray(inputs["edge_features"], np.float32)
    coords = np.asarray(inputs["edge_coords"], np.float32)
    mask = np.asarray(inputs["edge_mask"], np.float32)
    wq = np.asarray(inputs["wq"], np.float32)
    wk = np.asarray(inputs["wk"], np.float32)
    wv = np.asarray(inputs["wv"], np.float32)
    centers = np.asarray(inputs["rbf_centers"], np.float32)
    widths = np.asarray(inputs["rbf_widths"], np.float32)
    a_w1 = np.asarray(inputs["a_w1"], np.float32)
    a_b1 = np.asarray(inputs["a_b1"], np.float32)
    a_w2 = np.asarray(inputs["a_w2"], np.float32)
    a_b2 = np.asarray(inputs["a_b2"], np.float32)
    a_w3 = np.asarray(inputs["a_w3"], np.float32)
    g_w1 = np.asarray(inputs["g_w1"], np.float32)
    g_b1 = np.asarray(inputs["g_b1"], np.float32)
    g_w2 = np.asarray(inputs["g_w2"], np.float32)
    g_b2 = np.asarray(inputs["g_b2"], np.float32)
    wo = np.asarray(inputs["wo"], np.float32)
    bo = np.asarray(inputs["bo"], np.float32)
    ln_g = np.asarray(inputs["ln_g"], np.float32)
    ln_b = np.asarray(inputs["ln_b"], np.float32)

    if "nc" not in _CACHED:
        _CACHED["nc"] = build_program()
    nc = _CACHED["nc"]

    eye = np.eye(I, dtype=np.float32)

    rep = {
        "XT": np.ascontiguousarray(feat.T),
        "CT": np.ascontiguousarray(coords.T),
        "CE": coords,
        "WQ": wq, "WK": wk, "WV": wv,
        "W1Q": np.ascontiguousarray(a_w1[0:DH]),
        "W1K": np.ascontiguousarray(a_w1[DH:2 * DH]),
        "W1R": np.ascontiguousarray(a_w1[2 * DH:2 * DH + NR]),
        "W1D3": np.tile(a_w1[2 * DH + NR:2 * DH + NR + 1], (3, 1)),
        "CT3R": np.ascontiguousarray(coords.T),
        "B1C": a_b1.reshape(HID, 1),
        "W2": a_w2.astype(np.float32 if X1_FP32 else bf16np),
        "B2C": a_b2.reshape(HID, 1),
        "W3C": a_w3.astype(bf16np),
        "NEGC": -(centers - 1e-8).reshape(NR, 1),
        "NEGW": (-widths).reshape(NR, 1),
        "GW1": g_w1, "GB1": g_b1.reshape(HID, 1),
        "GW2": g_w2, "GB2": g_b2.reshape(1, 1),
        "WO": wo, "BOR": bo.reshape(1, HID),
        "LNG": ln_g.reshape(HID, 1), "LNB": ln_b.reshape(HID, 1),
        "EYE48": eye, "EYE48R": eye,
    }
    in_maps = []
    for c in range(NC):
        sl = slice(c * I, (c + 1) * I)
        m = dict(rep)
        m["XST"] = np.ascontiguousarray(feat[sl].T)
        m["FS"] = feat[sl]
        m["CS"] = np.ascontiguousarray(coords[sl].T)
        m["CSL"] = coords[sl]
        m["MASK"] = mask[sl]
        in_maps.append(m)

    from concourse.bass_utils import run_bass_kernel_spmd

    res = run_bass_kernel_spmd(nc, in_maps, list(range(NC)))
    out_f = np.concatenate([res.results[c]["OUTFT"].T for c in range(NC)], axis=0)
    coords_out = np.concatenate([res.results[c]["OUTC"] for c in range(NC)], axis=0)
    return out_f.astype(np.float32), coords_out.astype(np.float32)


# revision 53
# speedup vs baseline: 1.2628x; 1.0049x over previous
"""Trainium2 Bass kernel for EquivariantEdgeAttention (E=384, HID=128, NH=8).

Sharding: 8 cores, core c computes query-edge rows [48c, 48c+48) of the
pairwise attention; params / keys / values / coords are replicated.
Outputs are gathered by concatenation on the host. No collectives.

Self-contained: only numpy + concourse (bass) imports, no sibling files.
"""

import numpy as np

E = 384
NC = 8
I = E // NC          # 48 query rows per core
HID = 128
NH = 8
DH = 16
NR = 64
CUTOFF = 10.0

# ---- tuning config ----
B2 = 3               # i-block size in phase 2 (z2 psum banks per block)
B1 = 12              # i-block size in phase 1 (rbf/base build)
ACT_DT = None        # filled below (bf16) - dtype of x2 / W3diag path
BASE_BF16 = True     # store base (rbf+dot proj) as bf16
X1_FP32 = True       # x1 (L2 rhs) and W2 in fp32 (vs bf16)
MM_F32R = True      # use float32r for big fp32 matmuls


# --------------------------------------------------------------------------
# walrus workaround: this container's walrus rejects >1 sync wait on a CTRL
# Drain.  Split the TileContext tail drain into single-wait drains.
# --------------------------------------------------------------------------
def _patch_tile_drain():
    import concourse.tile as tile
    from concourse import mybir
    from concourse.vector_clock import ScopedClock

    def _drain_and_barrier_split(self, tick_clock, wait_clock):
        nc = self.nc
        drain_inst = nc.sync.drain()
        wait_clock.add_sem_waits(
            drain_inst.ins, ScopedClock({None: tick_clock.global_clock})
        )
        si = drain_inst.ins.sync_info
        waits = list(si.on_wait or [])
        if len(waits) > 1:
            drain_inst.ins.sync_info = mybir.SyncInfo(
                on_wait=waits[:1], on_update=list(si.on_update or [])
            )
            for w in waits[1:]:
                d2 = nc.sync.drain()
                d2.ins.sync_info = mybir.SyncInfo(on_wait=[w], on_update=[])
        nc.all_engine_barrier()
        assert self.sems is not None
        popped = nc._tile_sem_poison_stack.pop()
        assert popped is self._sem_poison
        nc.clear_and_free_semaphores(list(self.sems.allocated().values()))
        nc.all_engine_barrier()

    tile.TileContext._drain_and_barrier = _drain_and_barrier_split


def _split_excess_waits(nc, max_waits=1):
    """This container's walrus supports only one sync-wait command per
    engine instruction.  Move excess waits onto single-wait NoOps inserted
    immediately before the instruction on the same engine.  DMA-class
    instructions keep their waits (queue-mediated, not engine-decoded)."""
    from concourse import mybir

    n = 0
    for f in nc.m.functions:
        for bb in f.blocks:
            out = []
            for inst in bb.instructions:
                tn = type(inst).__name__
                si = inst.sync_info
                waits = list(si.on_wait) if si and si.on_wait else []
                if len(waits) > max_waits:
                    for w in waits[:-max_waits]:
                        nop = mybir.InstNoOp(
                            name=f"{inst.name}-w{n}", ins=[], outs=[],
                            engine=inst.engine)
                        nop.sync_info = mybir.SyncInfo(on_wait=[w], on_update=[])
                        out.append(nop)
                        n += 1
                    inst.sync_info = mybir.SyncInfo(
                        on_wait=waits[-max_waits:],
                        on_update=list(si.on_update or []))
                out.append(inst)
            bb.instructions = out
    return n


def build_program():
    import concourse.bass as bass
    import concourse.tile as tile
    from concourse import mybir
    from contextlib import ExitStack

    _patch_tile_drain()

    f32 = mybir.dt.float32
    bf16 = mybir.dt.bfloat16
    AF = mybir.ActivationFunctionType
    OP = mybir.AluOpType
    AX = mybir.AxisListType

    act_dt = bf16                      # x2 / W3diag dtype
    f32r = mybir.dt.float32r
    mm_f32 = f32r if MM_F32R else f32   # dtype of fp32 tensors feeding matmuls
    x1_dt = mm_f32 if X1_FP32 else bf16   # x1 / W2 dtype
    base_dt = bf16 if BASE_BF16 else f32

    nc = bass.Bass("TRN2", num_devices=NC)

    def din(name, shape, dt=f32):
        return nc.declare_dram_parameter(name, list(shape), dt, isOutput=False)

    # ---------------- dram inputs ----------------
    XT = din("XT", [HID, E], mm_f32)       # features^T (replicated)
    XST = din("XST", [HID, I], mm_f32)     # slab features^T
    FS = din("FS", [I, HID])              # slab features (residual)
    CT = din("CT", [3, E])                # coords^T
    CS = din("CS", [3, I])                # slab coords^T
    CE = din("CE", [E, 3])                # coords natural
    CSL = din("CSL", [I, 3])              # slab coords natural
    MASK = din("MASK", [I, E], mm_f32)
    EYE48R = din("EYE48R", [I, I], mm_f32)
    WQ = din("WQ", [HID, HID], mm_f32)
    WK = din("WK", [HID, HID], mm_f32)
    WV = din("WV", [HID, HID], mm_f32)
    W1Q = din("W1Q", [DH, HID], mm_f32)
    W1K = din("W1K", [DH, HID], mm_f32)
    W1R = din("W1R", [NR, HID], mm_f32)   # a_w1 rbf rows
    W1D3 = din("W1D3", [3, HID])          # a_w1 dot row replicated x3
    CT3R = din("CT3R", [3, E], mm_f32)    # coords^T for dot-term matmul
    B1C = din("B1C", [HID, 1])
    W2 = din("W2", [HID, HID], x1_dt)
    B2C = din("B2C", [HID, 1])
    W3C = din("W3C", [HID, NH], act_dt)   # a_w3
    NEGC = din("NEGC", [NR, 1])           # -(centers - 1e-8)
    NEGW = din("NEGW", [NR, 1])           # -widths
    GW1 = din("GW1", [DH, HID])
    GB1 = din("GB1", [HID, 1])
    GW2 = din("GW2", [HID, 1])
    GB2 = din("GB2", [1, 1])
    WO = din("WO", [HID, HID])
    BOR = din("BOR", [1, HID])
    LNG = din("LNG", [HID, 1])
    LNB = din("LNB", [HID, 1])
    EYE48 = din("EYE48", [I, I])

    OUTFT = nc.declare_dram_parameter("OUTFT", [HID, I], f32, isOutput=True)
    OUTC = nc.declare_dram_parameter("OUTC", [I, 3], f32, isOutput=True)

    DB_DRAM = nc.dram_tensor("db_scratch", [I, E], f32)

    def bcast_free(ap, n):
        """Insert a stride-0 dim of size n after the partition dim: [P, F] -> [P, n, F]."""
        return bass.AP(tensor=ap.tensor, offset=ap.offset,
                       ap=[ap.ap[0], [0, n]] + list(ap.ap[1:]))

    with tile.TileContext(nc) as tc, ExitStack() as ctx:
        singles = ctx.enter_context(tc.tile_pool(name="singles", bufs=1))

        def load(dram, shape, dt=f32, name=None):
            t = singles.tile(list(shape), dt, tag=name or dram.name)
            nc.sync.dma_start(out=t, in_=dram[:])
            return t

        # ---------------- load constants ----------------
        xt = load(XT, [HID, E], mm_f32)
        xst = load(XST, [HID, I], mm_f32)
        fs = load(FS, [I, HID])
        ct = load(CT, [3, E])
        cs = load(CS, [3, I])
        ce = []
        for c in range(3):
            t = singles.tile([128, 3], f32, tag=f"ce{c}")
            nc.sync.dma_start(out=t, in_=CE[c * 128:(c + 1) * 128, :])
            ce.append(t)
        csl = load(CSL, [I, 3])
        mask = load(MASK, [I, E], mm_f32)
        eye48r = load(EYE48R, [I, I], mm_f32)
        wq = load(WQ, [HID, HID], mm_f32)
        wk = load(WK, [HID, HID], mm_f32)
        wv = load(WV, [HID, HID], mm_f32)
        w1q = load(W1Q, [DH, HID], mm_f32)
        w1k = load(W1K, [DH, HID], mm_f32)
        w1r = load(W1R, [NR, HID], mm_f32)
        w1d3 = load(W1D3, [3, HID])
        ct3r = load(CT3R, [3, E], mm_f32)
        b1c = load(B1C, [HID, 1])
        w2 = load(W2, [HID, HID], x1_dt)
        b2c = load(B2C, [HID, 1])
        negc = load(NEGC, [NR, 1])
        negw = load(NEGW, [NR, 1])
        gw1 = load(GW1, [DH, HID])
        gb1 = load(GB1, [HID, 1])
        gw2 = load(GW2, [HID, 1])
        gb2 = load(GB2, [1, 1])
        one11 = singles.tile([1, 1], f32, tag="one11id")
        nc.vector.memset(one11, 1.0)
        wo = load(WO, [HID, HID])
        bor = load(BOR, [1, HID])
        lng = load(LNG, [HID, 1])
        lnb = load(LNB, [HID, 1])
        eye48 = load(EYE48, [I, I])
        w3c = load(W3C, [HID, NH], act_dt)
        w3diag = []
        for h in range(NH):
            t = singles.tile([HID, I, I], act_dt, name=f"w3diag{h}",
                             tag=f"w3diag{h}")
            nc.gpsimd.memset(t, 0.0)
            diag_view = bass.AP(tensor=t.tensor, offset=t.offset,
                                ap=[t.ap[0], [I + 1, I]])
            nc.vector.tensor_scalar(out=diag_view,
                                    in0=bcast_free(w3c[:, h:h + 1], I),
                                    scalar1=1.0, scalar2=None, op0=OP.mult)
            w3diag.append(t)

        ones31 = singles.tile([3, 1], f32)
        nc.vector.memset(ones31, 1.0)
        epsc = singles.tile([I, 1], f32)
        nc.vector.memset(epsc, 1e-5)

        # persistent intermediates
        base_sb = singles.tile([HID, I * E], base_dt, tag="base")
        kp1_sb = singles.tile([HID, NH, E], bf16, tag="kp1")
        qpb_sb = singles.tile([HID, NH, I], f32, tag="qpb")
        khead = [singles.tile([DH, E], mm_f32, name=f"khead{h}", tag=f"khead{h}") for h in range(NH)]
        v_ed = [singles.tile([HID, HID], f32, name=f"ved{c}", tag=f"ved{c}") for c in range(3)]
        gateT = singles.tile([I, NH], f32, tag="gateT")
        db_sb = singles.tile([I, E], f32, tag="db")
        f_sb = singles.tile([I, HID], f32, tag="fsb")
        cacc = singles.tile([I, 3], f32, tag="cacc")
        nc.vector.memset(cacc, 0.0)

        # ---------------- phase 0: projections & geometry ----------------
        with tc.tile_pool(name="p0psum", bufs=6, space="PSUM") as pp0, \
             tc.tile_pool(name="p0sb", bufs=4) as sp0:

            # v (all edges, natural layout): v_ed[c] = (X @ wv) rows chunk c
            for c in range(3):
                ps = pp0.tile([HID, HID], f32, tag="ps0")
                nc.tensor.matmul(ps, lhsT=xt[:, c * 128:(c + 1) * 128], rhs=wv)
                nc.vector.tensor_copy(out=v_ed[c], in_=ps)

            # per-head k rows:  khead[h] = (X @ wk)^T rows [16h:16h+16] = wk[:,h]^T X^T
            for h in range(NH):
                ps = pp0.tile([DH, E], f32, tag="ps0")
                nc.tensor.matmul(ps, lhsT=wk[:, h * DH:(h + 1) * DH], rhs=xt)
                nc.vector.tensor_copy(out=khead[h], in_=ps)
                # kp1 = W1k^T khead  [HID, E]
                ps2 = pp0.tile([HID, E], f32, tag="ps0")
                nc.tensor.matmul(ps2, lhsT=w1k, rhs=khead[h])
                nc.vector.tensor_copy(out=kp1_sb[:, h, :], in_=ps2)

            # per-head q rows + qpb = W1q^T qhead + b1
            for h in range(NH):
                ps = pp0.tile([DH, I], f32, tag="ps0")
                nc.tensor.matmul(ps, lhsT=wq[:, h * DH:(h + 1) * DH], rhs=xst)
                qh = sp0.tile([DH, I], mm_f32, tag="qh")
                nc.vector.tensor_copy(out=qh, in_=ps)
                ps2 = pp0.tile([HID, I], f32, tag="ps0")
                nc.tensor.matmul(ps2, lhsT=w1q, rhs=qh)
                nc.vector.tensor_scalar(out=qpb_sb[:, h, :], in0=ps2,
                                        scalar1=b1c, scalar2=None, op0=OP.add)

            # gates: sigmoid(silu(vh gw1 + gb1) gw2 + gb2), slab rows only.
            # All heads share one PSUM tile so each activation runs once.
            g1ps = pp0.tile([HID, NH, 64], f32, tag="g1ps", bufs=1)
            for h in range(NH):
                ps = pp0.tile([DH, I], f32, tag="ps0")
                nc.tensor.matmul(ps, lhsT=wv[:, h * DH:(h + 1) * DH], rhs=xst)
                vh = sp0.tile([DH, I], f32, tag="vh")
                nc.vector.tensor_copy(out=vh, in_=ps)
                nc.tensor.matmul(g1ps[:, h, 0:I], lhsT=gw1, rhs=vh,
                                 skip_group_check=True)
            g1all = sp0.tile([HID, NH, I], f32, tag="g1all")
            nc.scalar.activation(out=g1all, in_=g1ps[:, :, 0:I], func=AF.Silu,
                                 bias=gb1)
            g2ps = pp0.tile([1, NH, 64], f32, tag="g2ps", bufs=1)
            for h in range(NH):
                nc.tensor.matmul(g2ps[0:1, h, 0:I], lhsT=gw2, rhs=g1all[:, h, :],
                                 skip_group_check=True)
            gate_all = singles.tile([1, NH, I], f32, tag="gate_all")
            nc.scalar.activation(out=gate_all, in_=g2ps[:, :, 0:I],
                                 func=AF.Sigmoid, bias=gb2)
            for h in range(NH):
                psg = pp0.tile([I, 1], f32, tag="ps0")
                nc.tensor.transpose(psg, gate_all[0:1, h, :], one11)
                nc.vector.tensor_scalar(out=gateT[:, h:h + 1], in0=psg,
                                        scalar1=1.0 / NH, scalar2=None,
                                        op0=OP.mult)

            # geometry: D2 = -2 cs^T ct + 1 x n2row + n2slab (via psum accum)
            cs2 = sp0.tile([3, I], f32, tag="cs2")
            nc.vector.tensor_scalar(out=cs2, in0=cs, scalar1=-2.0, scalar2=None,
                                    op0=OP.mult)
            sq = sp0.tile([3, E], f32, tag="sq")
            nc.vector.tensor_mul(out=sq, in0=ct, in1=ct)
            psn = pp0.tile([1, E], f32, tag="ps0")
            nc.tensor.matmul(psn, lhsT=ones31, rhs=sq)
            n2row = sp0.tile([1, E], f32, tag="n2row")
            nc.vector.tensor_copy(out=n2row, in_=psn)
            sqs = sp0.tile([3, I], f32, tag="sqs")
            nc.vector.tensor_mul(out=sqs, in0=cs, in1=cs)
            psn2 = pp0.tile([I, 1], f32, tag="ps0")
            nc.tensor.matmul(psn2, lhsT=sqs, rhs=ones31)
            n2slab = sp0.tile([I, 1], f32, tag="n2slab")
            nc.vector.tensor_copy(out=n2slab, in_=psn2)
            ones1x48 = sp0.tile([1, I], f32, tag="ones1x48")
            nc.vector.memset(ones1x48, 1.0)

            psd2 = pp0.tile([I, E], f32, tag="ps0")
            nc.tensor.matmul(psd2, lhsT=cs2, rhs=ct, start=True, stop=False,
                             skip_group_check=True)
            nc.tensor.matmul(psd2, lhsT=ones1x48, rhs=n2row, start=False,
                             stop=True, skip_group_check=True)
            ds2 = sp0.tile([I, E], f32, tag="ds2")
            nc.vector.tensor_scalar(out=ds2, in0=psd2, scalar1=n2slab,
                                    scalar2=0.0, op0=OP.add, op1=OP.max)
            dd = sp0.tile([I, E], f32, tag="dd")
            nc.scalar.activation(out=dd, in_=ds2, func=AF.Sqrt)
            g9 = sp0.tile([I, E], f32, tag="g9")
            nc.vector.tensor_scalar(out=g9, in0=ds2, scalar1=CUTOFF * CUTOFF,
                                    scalar2=1e9, op0=OP.is_gt, op1=OP.mult)
            nc.vector.tensor_add(out=db_sb, in0=dd, in1=g9)
            nc.sync.dma_start(out=DB_DRAM[:], in_=db_sb)

        # ---------------- phase 1: base = W1rd^T [rbf; dot] ----------------
        with tc.tile_pool(name="p1psum", bufs=3, space="PSUM") as pp1, \
             tc.tile_pool(name="p1sb", bufs=4) as sp1:
            blk_sizes = [6, 6, 12, 12, 12]
            i0b = 0
            for blk, bsz in enumerate(blk_sizes):
                g4full = sp1.tile([NR, B1, E], mm_f32, name="g4full", tag="g4")
                g4 = g4full[:, 0:bsz, :]
                nc.sync.dma_start(
                    out=g4.bitcast(f32),
                    in_=DB_DRAM[i0b:i0b + bsz, :].partition_broadcast(NR))
                # (d - c)^2 then exp(-w * .), both on ScalarE (idle at ramp)
                nc.scalar.activation(out=g4, in_=g4, func=AF.Square,
                                     bias=negc)
                nc.scalar.activation(out=g4, in_=g4, func=AF.Exp, scale=negw)
                for u in range(bsz):
                    i = i0b + u
                    lhsd = sp1.tile([3, HID], mm_f32, tag="lhsd")
                    nc.vector.tensor_scalar(out=lhsd, in0=w1d3,
                                            scalar1=cs[:, i:i + 1], scalar2=None,
                                            op0=OP.mult)
                    ps = pp1.tile([HID, E], f32, tag="baseps")
                    nc.tensor.matmul(ps, lhsT=w1r, rhs=g4[:, u, :],
                                     start=True, stop=False, skip_group_check=True)
                    nc.tensor.matmul(ps, lhsT=lhsd, rhs=ct3r,
                                     start=False, stop=True, skip_group_check=True)
                    nc.any.tensor_copy(out=base_sb[:, i * E:(i + 1) * E], in_=ps)
                i0b += bsz

        # ---------------- phase 2: attention heads ----------------
        B6 = 6
        with tc.tile_pool(name="z2psum", bufs=2, space="PSUM") as zp, \
             tc.tile_pool(name="spsum", bufs=1, space="PSUM") as sp_pool, \
             tc.tile_pool(name="mpsum", bufs=1, space="PSUM") as mp, \
             tc.tile_pool(name="p2sb", bufs=4) as sb2:

            GH = 2
            pm_tiles = [singles.tile([I, GH, E], f32, name=f"pm{g}", tag=f"pm{g}")
                        for g in range(NH // GH)]
            pm_done = []
            for h in range(NH):
                s_ps = sp_pool.tile([I, E], f32, tag="S")
                for blk in range(I // B6):
                    i0 = blk * B6
                    z1 = sb2.tile([HID, B6, E], bf16, tag="z1", bufs=2)
                    nc.vector.tensor_add(
                        out=z1,
                        in0=base_sb[:, i0 * E:(i0 + B6) * E].rearrange(
                            "p (u e) -> p u e", u=B6),
                        in1=bcast_free(kp1_sb[:, h, :], B6))
                    for u in range(B6):
                        i = i0 + u
                        nc.vector.tensor_scalar(out=z1[:, u, :], in0=z1[:, u, :],
                                                scalar1=qpb_sb[:, h, i:i + 1],
                                                scalar2=None, op0=OP.add)
                    x1 = sb2.tile([HID, B6, E], x1_dt, tag="x1", bufs=2)
                    nc.scalar.activation(out=x1, in_=z1, func=AF.Silu)
                    for half in range(B6 // B2):
                        z2 = zp.tile([HID, B2, 512], f32, tag="z2")
                        for u3 in range(B2):
                            u = half * B2 + u3
                            nc.tensor.matmul(z2[:, u3, 0:E], lhsT=w2,
                                             rhs=x1[:, u, :])
                        x2 = sb2.tile([HID, B2, E], act_dt, tag="x2")
                        nc.scalar.activation(out=x2, in_=z2[:, :, 0:E],
                                             func=AF.Silu, bias=b2c)
                        for u3 in range(B2):
                            i = i0 + half * B2 + u3
                            nc.tensor.matmul(
                                s_ps, lhsT=w3diag[h][:, i, :], rhs=x2[:, u3, :],
                                start=(i == 0), stop=False, skip_group_check=True)
                # + mask
                nc.tensor.matmul(s_ps, lhsT=eye48r, rhs=mask, start=False,
                                 stop=True, skip_group_check=True)
                negmax = sb2.tile([I, 1], f32, tag="negmax")
                nc.vector.reduce_max(negmax, s_ps, axis=AX.X, negate=True)
                nc.vector.tensor_scalar(out=pm_tiles[h // GH][:, h % GH, :],
                                        in0=s_ps, scalar1=negmax, scalar2=None,
                                        op0=OP.add)
                if h % GH == GH - 1:
                    pms = sb2.tile([I, GH, E], f32, name=f"pms{h}", tag="pms")
                    nc.scalar.activation(out=pms, in_=pm_tiles[h // GH],
                                         func=AF.Exp)
                    pm_done.append(pms)

            # ------------- phase 2b: softmax + attn application -------------
            for h in range(NH):
                p_sb = pm_done[h // GH][:, h % GH, :]
                rs = sb2.tile([I, 1], f32, tag="rs")
                nc.vector.reduce_sum(rs, p_sb, axis=AX.X)
                rinv = sb2.tile([I, 1], f32, tag="rinv")
                nc.vector.reciprocal(rinv, rs)
                a_sb = sb2.tile([I, E], f32, tag="a")
                nc.vector.tensor_scalar(out=a_sb, in0=p_sb, scalar1=rinv,
                                        scalar2=None, op0=OP.mult)

                at = [sb2.tile([128, I], f32, name=f"at{c}", tag=f"at{c}")
                      for c in range(3)]
                for c in range(3):
                    pst = mp.tile([128, I], f32, tag="mps")
                    nc.tensor.transpose(pst, a_sb[:, c * 128:(c + 1) * 128], eye48)
                    nc.vector.tensor_copy(out=at[c], in_=pst)
                fps = mp.tile([I, DH], f32, tag="mps")
                for c in range(3):
                    nc.tensor.matmul(fps, lhsT=at[c],
                                     rhs=v_ed[c][:, h * DH:(h + 1) * DH],
                                     start=(c == 0), stop=(c == 2))
                nc.vector.tensor_copy(out=f_sb[:, h * DH:(h + 1) * DH], in_=fps)
                cps = mp.tile([I, 3], f32, tag="mps")
                for c in range(3):
                    nc.tensor.matmul(cps, lhsT=at[c], rhs=ce[c],
                                     start=(c == 0), stop=(c == 2))
                cw = sb2.tile([I, 3], f32, tag="cw")
                nc.vector.tensor_scalar(out=cw, in0=cps,
                                        scalar1=gateT[:, h:h + 1], scalar2=None,
                                        op0=OP.mult)
                nc.vector.tensor_add(out=cacc, in0=cacc, in1=cw)

            # ---------------- epilogue ----------------
            ftps = mp.tile([HID, I], f32, tag="mps")
            nc.tensor.transpose(ftps, f_sb, eye48)
            ft = sb2.tile([HID, I], f32, tag="ft")
            nc.vector.tensor_copy(out=ft, in_=ftps)
            yps = mp.tile([I, HID], f32, tag="mps")
            nc.tensor.matmul(yps, lhsT=ft, rhs=wo, start=True, stop=False,
                             skip_group_check=True)
            nc.tensor.matmul(yps, lhsT=eye48, rhs=fs, start=False, stop=False,
                             skip_group_check=True)
            ones1I = sb2.tile([1, I], f32, tag="ones1I")
            nc.vector.memset(ones1I, 1.0)
            nc.tensor.matmul(yps, lhsT=ones1I, rhs=bor, start=False, stop=True,
                             skip_group_check=True)

            stats = sb2.tile([I, 6], f32, tag="stats")
            nc.vector.bn_stats(out=stats, in_=yps)
            mv = sb2.tile([I, 2], f32, tag="mv")
            nc.vector.bn_aggr(out=mv, in_=stats)
            sd = sb2.tile([I, 1], f32, tag="sd")
            nc.scalar.activation(out=sd, in_=mv[:, 1:2], func=AF.Sqrt,
                                 bias=epsc)
            rstd = sb2.tile([I, 1], f32, tag="rstd")
            nc.vector.reciprocal(rstd, sd)
            yc = sb2.tile([I, HID], f32, tag="yc")
            nc.vector.tensor_scalar(out=yc, in0=yps, scalar1=mv[:, 0:1],
                                    scalar2=rstd, op0=OP.subtract, op1=OP.mult)
            ytps = mp.tile([HID, I], f32, tag="mps")
            nc.tensor.transpose(ytps, yc, eye48)
            outt = sb2.tile([HID, I], f32, tag="outt")
            nc.vector.tensor_scalar(out=outt, in0=ytps, scalar1=lng, scalar2=lnb,
                                    op0=OP.mult, op1=OP.add)
            nc.sync.dma_start(out=OUTFT[:], in_=outt)

            # coords_out = csl + sumg * csl_xi - cacc  (sumg = sum_h gate/NH)
            sumg = sb2.tile([I, 1], f32, tag="sumg")
            nc.vector.reduce_sum(sumg, gateT, axis=AX.X)
            own = sb2.tile([I, 3], f32, tag="own")
            nc.vector.tensor_scalar(out=own, in0=csl, scalar1=sumg, scalar2=None,
                                    op0=OP.mult)
            co = sb2.tile([I, 3], f32, tag="co")
            nc.vector.tensor_add(out=co, in0=csl, in1=own)
            nc.vector.tensor_sub(out=co, in0=co, in1=cacc)
            nc.sync.dma_start(out=OUTC[:], in_=co)

    _split_excess_waits(nc)
    return nc


_CACHED = {}


def kernel(**inputs):
    import ml_dtypes

    bf16np = ml_dtypes.bfloat16

    feat = np.asarray(inputs["edge_features"], np.float32)
    coords = np.asarray(inputs["edge_coords"], np.float32)
    mask = np.asarray(inputs["edge_mask"], np.float32)
    wq = np.asarray(inputs["wq"], np.float32)
    wk = np.asarray(inputs["wk"], np.float32)
    wv = np.asarray(inputs["wv"], np.float32)
    centers = np.asarray(inputs["rbf_centers"], np.float32)
    widths = np.asarray(inputs["rbf_widths"], np.float32)
    a_w1 = np.asarray(inputs["a_w1"], np.float32)
    a_b1 = np.asarray(inputs["a_b1"], np.float32)
    a_w2 = np.asarray(inputs["a_w2"], np.float32)
    a_b2 = np.asarray(inputs["a_b2"], np.float32)
    a_w3 = np.asarray(inputs["a_w3"], np.float32)
    g_w1 = np.asarray(inputs["g_w1"], np.float32)
    g_b1 = np.asarray(inputs["g_b1"], np.float32)
    g_w2 = np.asarray(inputs["g_w2"], np.float32)
    g_b2 = np.asarray(inputs["g_b2"], np.float32)
    wo = np.asarray(inputs["wo"], np.float32)
    bo = np.asarray(inputs["bo"], np.float32)
    ln_g = np.asarray(inputs["ln_g"], np.float32)
    ln_b = np.asarray(inputs["ln_b"], np.float32)

    if "nc" not in _CACHED:
        _CACHED["nc"] = build_program()
    nc = _CACHED["nc"]

    eye = np.eye(I, dtype=np.float32)

    rep = {
        "XT": np.ascontiguousarray(feat.T),
        "CT": np.ascontiguousarray(coords.T),
        "CE": coords,
        "WQ": wq, "WK": wk, "WV": wv,
        "W1Q": np.ascontiguousarray(a_w1[0:DH]),
        "W1K": np.ascontiguousarray(a_w1[DH:2 * DH]),
        "W1R": np.ascontiguousarray(a_w1[2 * DH:2 * DH + NR]),
        "W1D3": np.tile(a_w1[2 * DH + NR:2 * DH + NR + 1], (3, 1)),
        "CT3R": np.ascontiguousarray(coords.T),
        "B1C": a_b1.reshape(HID, 1),
        "W2": a_w2.astype(np.float32 if X1_FP32 else bf16np),
        "B2C": a_b2.reshape(HID, 1),
        "W3C": a_w3.astype(bf16np),
        "NEGC": -(centers - 1e-8).reshape(NR, 1),
        "NEGW": (-widths).reshape(NR, 1),
        "GW1": g_w1, "GB1": g_b1.reshape(HID, 1),
        "GW2": g_w2, "GB2": g_b2.reshape(1, 1),
        "WO": wo, "BOR": bo.reshape(1, HID),
        "LNG": ln_g.reshape(HID, 1), "LNB": ln_b.reshape(HID, 1),
        "EYE48": eye, "EYE48R": eye,
    }
    in_maps = []
    for c in range(NC):
        sl = slice(c * I, (c + 1) * I)
        m = dict(rep)
        m["XST"] = np.ascontiguousarray(feat[sl].T)
        m["FS"] = feat[sl]
        m["CS"] = np.ascontiguousarray(coords[sl].T)
        m["CSL"] = coords[sl]
        m["MASK"] = mask[sl]
        in_maps.append(m)

    from concourse.bass_utils import run_bass_kernel_spmd

    res = run_bass_kernel_spmd(nc, in_maps, list(range(NC)))
    out_f = np.concatenate([res.results[c]["OUTFT"].T for c in range(NC)], axis=0)
    coords_out = np.concatenate([res.results[c]["OUTC"] for c in range(NC)], axis=0)
    return out_f.astype(np.float32), coords_out.astype(np.float32)
